# revision 15
# baseline (speedup 1.0000x reference)
"""LSTM caption-decoder kernel for 8 trn2 NeuronCores (Bass/Tile, SPMD).

Sharding: data-parallel over batch, 16 rows per core, dealt round-robin from
the length-sorted order so every core sees the same live-row profile.

Per-core structure (v2 — col-tiled recurrence + ragged fc):
  - gates are computed with 4x column-tiled matmuls: the PE array's four
    32-column groups concurrently produce the four hidden-quarters of the
    (gate-interleaved) 2048 gate columns, so one step costs ~5 rounds of
    512 cycles instead of 20 serial matmuls.  Gate columns are permuted
    host-side to [i|f|o|g] per hidden quarter, so group g's PSUM rows
    [32g:32g+16] hold all four gate types for hidden quarter g and every
    elementwise op in the LSTM tail is partition-aligned and [128, *]-shaped.
  - tail ops are spread across ACT (sigmoid/tanh), DVE (muls) and Pool
    (adds/copies) so no single engine serializes the step.
  - h is produced bf16, PE-transposed per quarter into one [128, 64] PSUM
    tile, copied once into a double-buffered hprevT (next step's lhsT), and
    its live-prefix columns are packed into hall for the fc.
  - fc runs transposed (vocab tiles on partitions, packed live (t,b) columns
    streamed): only ~sum(len)/8 columns per core are projected, bias and
    length-masking are applied host-side, and windows of hall columns are
    projected as soon as the recurrence has produced them.
"""

import sys
import os

if "/opt/trn_rl_repo" not in sys.path:
    sys.path.insert(0, "/opt/trn_rl_repo")

import numpy as np
import ml_dtypes

BF = ml_dtypes.bfloat16

B, T, E, H, V, LF = 128, 32, 512, 512, 10000, 49
NCORES = 8
BS = B // NCORES          # 16 batch rows per core
GC = 4 * H                # 2048 gate columns
KT = 4                    # k tiles (H/128)
NW = T // 8               # 4 wx windows of 128 (t,b) rows
VCH = 500                 # vocab chunk (psum bank limit for fp32)
NVC = V // VCH            # 20 vocab chunks
FCG = 5                   # fc chunks per output DMA group
CB = 3600                 # packed bf16 const blob cols

# gate column permutation (applied to torch's [i|f|g|o] stacking):
# new col 512*q + 128*blk + x  <-  torch col srcblk[blk]*512 + 128*q + x
# with block order [i, f, o, g] inside each hidden-quarter q.
_SRC = (0, 1, 3, 2)       # i, f, o, g

def _perm_new():
    p = np.empty(GC, np.int64)
    for q in range(4):
        for b in range(4):
            x = np.arange(128)
            p[512 * q + 128 * b + x] = 512 * _SRC[b] + 128 * q + x
    return p

_PERM = _perm_new()

_CACHE: dict = {}


def _schedule(lengths):
    """Uniform per-step live-prefix schedule from the (sorted) lengths."""
    L = np.sort(np.asarray(lengths).astype(np.int64))[::-1]
    order = np.argsort(-np.asarray(lengths).astype(np.int64), kind="stable")
    # core c slot j holds original row order[8j + c]; P[t] is the uniform
    # (max-over-cores) count of slots with len > t = count over j of L[8j]>t
    P = [int(np.sum(L[0::NCORES] > t)) for t in range(T)]
    pos = np.concatenate([[0], np.cumsum(P)]).astype(np.int64)
    nfc = int(pos[-1])
    nwin = -(-nfc // 128)
    wtot = 128 * nwin
    # ready[w]: first step index t after which hall cols < min(128(w+1), nfc)
    # are all written
    ready = []
    for w in range(nwin):
        wend = min(128 * (w + 1), nfc)
        t = next(t for t in range(T) if pos[t + 1] >= wend)
        ready.append(t)
    # spread each window's vocab chunks across steps ready+1 .. T-1
    fc_at_step = [[] for _ in range(T)]
    fc_tail = []
    for w, r in enumerate(ready):
        steps = list(range(r + 1, T))
        if not steps:
            fc_tail.extend((w, nv) for nv in range(NVC))
            continue
        per = -(-NVC // len(steps))
        nv = 0
        for s in steps:
            for _ in range(per):
                if nv < NVC:
                    fc_at_step[s].append((w, nv))
                    nv += 1
        while nv < NVC:
            fc_tail.append((w, nv))
            nv += 1
    return {
        "order": order, "P": P, "pos": pos, "nfc": nfc, "wtot": wtot,
        "fc_at_step": fc_at_step, "fc_tail": fc_tail,
    }


def _emit(nc, tc, tile, bass, mybir, d, sched, rep=1):
    for r in range(rep):
        _emit_once(nc, tc, tile, bass, mybir, d, sched,
                   str(r) if rep > 1 else "")


def _emit_once(nc, tc, tile, bass, mybir, d, sched, pfx=""):
    from contextlib import ExitStack

    dt = mybir.dt
    f32, bf, i32 = dt.float32, dt.bfloat16, dt.int32
    AF = mybir.ActivationFunctionType
    AL = mybir.AluOpType

    P, pos, nfc = sched["P"], sched["pos"], sched["nfc"]
    wtot = sched["wtot"]

    ctx = ExitStack()
    with ctx:
        psp = ctx.enter_context(tc.tile_pool(name="ps" + pfx, bufs=1,
                                             space="PSUM"))
        cp = ctx.enter_context(tc.tile_pool(name="const" + pfx, bufs=1))
        fcp = ctx.enter_context(tc.tile_pool(name="fcp" + pfx, bufs=1))
        wp = ctx.enter_context(tc.tile_pool(name="w" + pfx, bufs=1))
        sp = ctx.enter_context(tc.tile_pool(name="state" + pfx, bufs=1))
        wk = ctx.enter_context(tc.tile_pool(name="work" + pfx, bufs=2))
        fout = ctx.enter_context(tc.tile_pool(name="fout" + pfx, bufs=2))

        # ---- the big fc weight load starts first on the sync queue so it
        # streams in behind the init phase
        fcwa = fcp.tile([128, KT * V], bf)
        nc.sync.dma_start(fcwa[:].rearrange("p (k n) -> p k n", k=KT),
                          d["fcw"].rearrange("(k p) n -> p k n", k=KT))
        fcw = [fcwa[:, k * V:(k + 1) * V] for k in range(KT)]

        bfc = cp.tile([128, CB], bf)
        nc.gpsimd.dma_start(bfc[:], d["bfconst"])
        id128 = bfc[:, 0:128]
        id16g = [bfc[32 * k:32 * k + BS, 128:144] for k in range(4)]
        gbias = bfc[0:1, 144:144 + GC]
        onesr = bfc[0:1, 3216:3216 + 128]
        initbh = bfc[:, 3344:3472]
        initbc = bfc[:, 3472:3600]

        whha = wp.tile([128, KT * GC], bf)
        nc.gpsimd.dma_start(whha[:].rearrange("p (k n) -> p k n", k=KT),
                            d["whh"].rearrange("(k p) n -> p k n", k=KT))
        whh = [whha[:, k * GC:(k + 1) * GC] for k in range(KT)]

        # ---- persistent state
        # gates PSUM: two persistent buffers, gap partitions zeroed once so
        # full-[128,*] tail reads see initialized memory
        gbuf = [psp.tile([128, 512], f32, tag=f"g{i}", bufs=1,
                         name=f"gbuf{i}") for i in range(2)]
        nc.vector.memset(gbuf[0][:], 0.0)
        nc.vector.memset(gbuf[1][:], 0.0)
        hall = [sp.tile([128, wtot], bf, name=f"hall{k}")
                for k in range(KT)]
        if wtot > nfc:
            for k in range(KT):
                nc.gpsimd.memset(hall[k][:, nfc:wtot], 0.0)
        wx = [sp.tile([128, GC], bf, name=f"wx{m}") for m in range(NW)]
        c_st = [sp.tile([128, 128], f32, name=f"c{i}") for i in range(2)]
        hpT = [sp.tile([128, 128], bf, name=f"hpT{i}") for i in range(2)]

        # ================= init phase (transient pool) =================
        with tc.tile_pool(name="init" + pfx, bufs=1) as ip:
            mfs = ip.tile([BS, E], bf)
            nc.gpsimd.dma_start(mfs[:], d["mf"])
            wiha = ip.tile([128, KT * GC], bf)
            nc.gpsimd.dma_start(wiha[:].rearrange("p (k n) -> p k n", k=KT),
                                d["wih"].rearrange("(k p) n -> p k n", k=KT))
            wih = [wiha[:, k * GC:(k + 1) * GC] for k in range(KT)]
            initwa = ip.tile([128, KT * 2 * H], bf)
            nc.gpsimd.dma_start(initwa[:].rearrange("p (k n) -> p k n", k=KT),
                                d["initw"].rearrange("(k p) n -> p k n", k=KT))

            # mf^T tiles [128, 16]
            mfT = [ip.tile([128, BS], bf, name=f"mfT{k}") for k in range(KT)]
            for k in range(KT):
                tp = psp.tile([128, 128], bf, tag="ht", bufs=2)
                nc.tensor.transpose(tp[:, 0:BS], mfs[:, 128 * k:128 * (k + 1)],
                                    id16g[0])
                nc.vector.tensor_copy(mfT[k][:], tp[:, 0:BS])

            # h0 | c0 via col-tiled matmuls: group g rows [32g:32g+16],
            # free [0:128]=h0 quarter g, [128:256]=c0 quarter g
            hc = gbuf[0]
            for g in range(4):
                gsl = hc[32 * g:32 * g + BS, :]
                for half in range(2):
                    out = gsl[:, 128 * half:128 * half + 128]
                    for k in range(KT):
                        off = k * 2 * H + 512 * half + 128 * g
                        nc.tensor.matmul(
                            out, lhsT=mfT[k][:],
                            rhs=initwa[:, off:off + 128],
                            start=(half == 0 and k == 0),
                            stop=(half == 1 and k == KT - 1),
                            tile_position=(0, 32 * g),
                            skip_group_check=True)
            nc.vector.tensor_add(c_st[0][:], hc[:, 128:256], initbc)
            h0sb = ip.tile([128, 128], bf)
            nc.vector.tensor_add(h0sb[:], hc[:, 0:128], initbh)
            htp0 = psp.tile([128, 128], bf, tag="ht", bufs=2)
            nc.tensor.transpose(htp0[:], h0sb[:], id128[:])
            nc.vector.tensor_copy(hpT[0][:], htp0[:])

            # embedding gather (indirect DMA), rows in (t,slot) order
            idxc = ip.tile([128, NW], i32)
            nc.sync.dma_start(idxc[:], d["idx"])
            embm = [ip.tile([128, E], bf, name=f"embm{m}") for m in range(NW)]
            for m in range(NW):
                nc.gpsimd.indirect_dma_start(
                    out=embm[m][:], out_offset=None,
                    in_=d["embtab"],
                    in_offset=bass.IndirectOffsetOnAxis(ap=idxc[:, m:m + 1],
                                                        axis=0),
                )
            embT = [ip.tile([128, 128 * NW], bf, name=f"embT{k}")
                    for k in range(KT)]
            for m in range(NW):
                for k in range(KT):
                    tp = psp.tile([128, 128], bf, tag="ht", bufs=2)
                    nc.tensor.transpose(tp[:], embm[m][:, 128 * k:128 * (k + 1)],
                                        id128[:])
                    nc.vector.tensor_copy(embT[k][:, 128 * m:128 * (m + 1)],
                                          tp[:])

            # Wx = emb @ W_ih^T + gbias  -> wx[m] [128, GC] bf16
            for m in range(NW):
                for nch in range(4):
                    sl = slice(512 * nch, 512 * (nch + 1))
                    wps = psp.tile([128, 512], f32, tag="fc", bufs=2)
                    for k in range(KT):
                        nc.tensor.matmul(
                            wps[:], lhsT=embT[k][:, 128 * m:128 * (m + 1)],
                            rhs=wih[k][:, sl], start=(k == 0), stop=False)
                    nc.tensor.matmul(wps[:], lhsT=onesr[:], rhs=gbias[:, sl],
                                     start=False, stop=True)
                    if nch % 2 == 0:
                        nc.scalar.copy(wx[m][:, sl], wps[:])
                    else:
                        nc.vector.tensor_copy(wx[m][:, sl], wps[:])

        # ---- fc emission helper --------------------------------------
        dma_engs = [nc.sync, nc.gpsimd]
        fc_state = {"osb": None, "nv0": None, "w": None, "n": 0, "ndma": 0}

        def fc_flush():
            st = fc_state
            if st["osb"] is None:
                return
            g = st["n"]
            rows = min(128, nfc - 128 * st["w"])
            dst = d["preds"][128 * st["w"]:128 * st["w"] + rows,
                             VCH * st["nv0"]:VCH * (st["nv0"] + g)]
            eng = dma_engs[st["ndma"] % 2]
            eng.dma_start(dst, st["osb"][0:rows, 0:g * VCH])
            st["ndma"] += 1
            st["osb"] = None
            st["n"] = 0

        def fc_do(w, nv):
            st = fc_state
            if st["osb"] is not None and (st["w"] != w or st["n"] == FCG
                                          or nv != st["nv0"] + st["n"]):
                fc_flush()
            if st["osb"] is None:
                st["osb"] = fout.tile([128, FCG * VCH], bf, tag="fo",
                                      name="osb")
                st["nv0"] = nv
                st["w"] = w
            fps = psp.tile([128, VCH], f32, tag="fc", bufs=2)
            vsl = slice(VCH * nv, VCH * (nv + 1))
            for k in range(KT):
                nc.tensor.matmul(fps[:],
                                 lhsT=hall[k][:, 128 * w:128 * (w + 1)],
                                 rhs=fcw[k][:, vsl],
                                 start=(k == 0), stop=(k == KT - 1))
            osl = st["osb"][:, st["n"] * VCH:(st["n"] + 1) * VCH]
            if nv % 2 == 0:
                nc.scalar.copy(osl, fps[:])
            else:
                nc.vector.tensor_copy(osl, fps[:])
            st["n"] += 1
            if st["n"] == FCG:
                fc_flush()

        # ================= recurrence =================
        def selector(t):
            m, j = t // 8, t % 8
            g_ps = gbuf[t % 2]
            for g in range(4):
                nc.tensor.matmul(g_ps[32 * g:32 * g + BS, :],
                                 lhsT=id128[:, 16 * j:16 * (j + 1)],
                                 rhs=wx[m][:, 512 * g:512 * (g + 1)],
                                 start=True, stop=False,
                                 tile_position=(0, 32 * g),
                                 skip_group_check=True)

        selector(0)
        for t in range(T):
            hp_r, hp_w = hpT[t % 2], hpT[(t + 1) % 2]
            g_ps = gbuf[t % 2]
            for k in range(KT):
                for g in range(4):
                    nc.tensor.matmul(g_ps[32 * g:32 * g + BS, :],
                                     lhsT=hp_r[:, 32 * k:32 * k + BS],
                                     rhs=whh[k][:, 512 * g:512 * (g + 1)],
                                     start=False, stop=(k == KT - 1),
                                     tile_position=(0, 32 * g),
                                     skip_group_check=True)
            # tail: [i|f|o|g] per group; all ops partition-aligned.
            # tanh(g) and sigmoid(i) first so p1 starts as early as possible;
            # the c-chain stays on DVE to avoid cross-engine sem hops.
            tg = wk.tile([128, 128], f32, tag="tg")
            nc.scalar.activation(tg[:], g_ps[:, 384:512], AF.Tanh)
            sig = wk.tile([128, 384], f32, tag="sig")
            nc.scalar.activation(sig[:, 0:128], g_ps[:, 0:128], AF.Sigmoid)
            nc.scalar.activation(sig[:, 128:256], g_ps[:, 128:256],
                                 AF.Sigmoid)
            p1 = wk.tile([128, 128], f32, tag="p1")
            nc.vector.tensor_mul(p1[:], sig[:, 0:128], tg[:])
            p2 = wk.tile([128, 128], f32, tag="p2")
            nc.vector.tensor_mul(p2[:], sig[:, 128:256], c_st[t % 2][:])
            c_new = c_st[(t + 1) % 2]
            nc.vector.tensor_add(c_new[:], p1[:], p2[:])
            nc.scalar.activation(sig[:, 256:384], g_ps[:, 256:384],
                                 AF.Sigmoid)
            tnc = wk.tile([128, 128], f32, tag="tnc")
            nc.scalar.activation(tnc[:], c_new[:], AF.Tanh)
            h_sb = wk.tile([128, 128], bf, tag="h")
            nc.vector.tensor_mul(h_sb[:], tnc[:], sig[:, 256:384])
            # fc work for this step sits between the whh matmuls and the
            # transpose in the PE stream, so the PE keeps busy through the
            # elementwise tail instead of stalling on the blocked transpose
            for (w, nv) in sched["fc_at_step"][t]:
                fc_do(w, nv)
            if t + 1 < T:
                selector(t + 1)
            htp = psp.tile([128, 128], bf, tag="ht", bufs=2, name="htp")
            nc.tensor.transpose(htp[:], h_sb[:], id128[:])
            # split the hprevT copy so next step's k=0 matmul can begin
            # before the full 128 columns have landed
            nc.vector.tensor_copy(hp_w[:, 0:32], htp[:, 0:32])
            nc.vector.tensor_copy(hp_w[:, 32:128], htp[:, 32:128])
            if P[t] > 0:
                for k in range(KT):
                    dstc = hall[k][:, pos[t]:pos[t] + P[t]]
                    srcc = hp_w[:, 32 * k:32 * k + P[t]]
                    nc.gpsimd.tensor_copy(dstc, srcc)
        for (w, nv) in sched["fc_tail"]:
            fc_do(w, nv)
        fc_flush()


def _build(lengths, rep=1):
    sched = _schedule(lengths)
    key = ("nc", rep, sched["nfc"], tuple(sched["P"]))
    if key in _CACHE:
        return _CACHE[key]
    import concourse.bass as bass
    import concourse.tile as tile
    from concourse import bacc, mybir

    dt = mybir.dt
    nc = bacc.Bacc("TRN2", target_bir_lowering=False, debug=False,
                   num_devices=NCORES)

    def din(name, shape, dty):
        return nc.dram_tensor(name, shape, dty, kind="ExternalInput").ap()

    d = {
        "embtab": din("embtab", [V, E], dt.bfloat16),
        "idx": din("idx", [128, NW], dt.int32),
        "wih": din("wih", [E, GC], dt.bfloat16),
        "whh": din("whh", [H, GC], dt.bfloat16),
        "initw": din("initw", [E, 2 * H], dt.bfloat16),
        "fcw": din("fcw", [H, V], dt.bfloat16),
        "mf": din("mf", [BS, E], dt.bfloat16),
        "bfconst": din("bfconst", [128, CB], dt.bfloat16),
        "preds": nc.dram_tensor("preds", [sched["wtot"], V],
                                dt.bfloat16, kind="ExternalOutput").ap(),
    }

    with tile.TileContext(nc) as tc:
        _emit(nc, tc, tile, bass, mybir, d, sched, rep=rep)
    nc.compile()
    _CACHE[key] = (nc, sched)
    return nc, sched


def _shared_inputs(embedding, W_ih, W_hh, b_ih, b_hh, fc_w, fc_b,
                   init_h_w, init_h_b, init_c_w, init_c_b):
    sh = {}
    sh["embtab"] = np.ascontiguousarray(embedding, dtype=np.float32).astype(BF)
    sh["wih"] = np.ascontiguousarray(np.asarray(W_ih)[_PERM].T).astype(BF)
    sh["whh"] = np.ascontiguousarray(np.asarray(W_hh)[_PERM].T).astype(BF)
    iw = np.concatenate([init_h_w, init_c_w], axis=0)  # [2H, 512]
    sh["initw"] = np.ascontiguousarray(iw.T).astype(BF)
    sh["fcw"] = np.ascontiguousarray(np.asarray(fc_w).T).astype(BF)
    blob = np.zeros((128, CB), dtype=BF)
    blob[:, 0:128] = np.eye(128, dtype=np.float32).astype(BF)
    for k in range(4):
        blob[32 * k:32 * k + BS, 128:144] = np.eye(BS, dtype=np.float32).astype(BF)
    blob[0, 144:144 + GC] = (np.asarray(b_ih) + np.asarray(b_hh))[_PERM].astype(BF)
    blob[0, 3216:3216 + 128] = np.ones(128, np.float32).astype(BF)
    bh = np.asarray(init_h_b, np.float32).reshape(4, 128)
    bc = np.asarray(init_c_b, np.float32).reshape(4, 128)
    for g in range(4):
        blob[32 * g:32 * g + 32, 3344:3472] = bh[g].astype(BF)
        blob[32 * g:32 * g + 32, 3472:3600] = bc[g].astype(BF)
    sh["bfconst"] = blob
    return sh


def _core_inputs(sh, features, captions, order, ci):
    rows = np.asarray(order[ci::NCORES])        # slots j=0..15
    cap = np.asarray(captions)[rows].astype(np.int64)   # [16, T]
    m = dict(sh)
    mf = np.asarray(features, np.float32)[rows].mean(axis=1)  # [16, 512]
    m["mf"] = mf.astype(BF)
    m["idx"] = np.ascontiguousarray(
        cap.T.reshape(NW, 128).T).astype(np.int32)
    return m


def _in_maps(inputs, sched):
    sh = _shared_inputs(
        inputs["embedding"], inputs["W_ih"], inputs["W_hh"], inputs["b_ih"],
        inputs["b_hh"], inputs["fc_w"], inputs["fc_b"], inputs["init_h_w"],
        inputs["init_h_b"], inputs["init_c_w"], inputs["init_c_b"])
    return [
        _core_inputs(sh, inputs["features"], inputs["captions"],
                     sched["order"], ci)
        for ci in range(NCORES)
    ]


def _postprocess(raw, inputs, sched):
    """raw: list of per-core [VP, nfc] arrays -> full [B, T, V] fp32."""
    lengths = np.asarray(inputs["lengths"]).astype(np.int64)
    fc_b = np.asarray(inputs["fc_b"], np.float32)
    order, P, pos = sched["order"], sched["P"], sched["pos"]
    nfc = sched["nfc"]
    col_t = np.empty(nfc, np.int64)
    col_j = np.empty(nfc, np.int64)
    for t in range(T):
        col_t[pos[t]:pos[t + 1]] = t
        col_j[pos[t]:pos[t + 1]] = np.arange(P[t])
    preds = np.zeros((B, T, V), np.float32)
    for ci in range(NCORES):
        rows = order[ci::NCORES]
        rb = rows[col_j]                       # original batch row per col
        live = lengths[rb] > col_t
        arr = np.asarray(raw[ci][:nfc, :], dtype=np.float32)
        preds[rb[live], col_t[live], :] = arr[live, :] + fc_b
    return preds


def _run(inputs, trace=False):
    from concourse.bass_utils import run_bass_kernel_spmd
    nc, sched = _build(inputs["lengths"])
    res = run_bass_kernel_spmd(nc, _in_maps(inputs, sched),
                               list(range(NCORES)), trace=trace)
    raw = [np.asarray(r["preds"]) for r in res.results]
    return _postprocess(raw, inputs, sched), res


def kernel(**inputs):
    """Run on HW. The first execution after a fresh NEFF compile has been
    observed to crash the exec unit sporadically (and poison the in-process
    jax runtime), so the device run happens in a subprocess with retries."""
    if os.environ.get("_LSTM_KERNEL_CHILD"):
        preds, _ = _run(inputs, trace=False)
        return preds
    import subprocess
    import tempfile
    import pickle
    with tempfile.TemporaryDirectory() as td:
        fin = os.path.join(td, "in.pkl")
        fout_p = os.path.join(td, "out.npy")
        with open(fin, "wb") as f:
            pickle.dump({k: np.asarray(v) for k, v in inputs.items()}, f)
        code = (
            "import pickle,numpy as np,sys;"
            f"sys.path.insert(0,{os.path.dirname(os.path.abspath(__file__))!r});"
            "import kernel;"
            f"ins=pickle.load(open({fin!r},'rb'));"
            f"np.save({fout_p!r}, kernel.kernel(**ins))"
        )
        env = {**os.environ, "_LSTM_KERNEL_CHILD": "1"}
        last = None
        for attempt in range(3):
            r = subprocess.run([sys.executable, "-c", code], env=env,
                               capture_output=True, text=True)
            if r.returncode == 0 and os.path.exists(fout_p):
                return np.load(fout_p)
            last = r
        raise RuntimeError(
            f"kernel subprocess failed after retries:\n{last.stdout[-2000:]}"
            f"\n{last.stderr[-4000:]}")


def _timed_runner(nc, in_maps):
    """Build the same shard_map executable run_bass_via_pjrt uses, but keep it
    for repeated timed execution with device-resident inputs."""
    import jax
    from jax.sharding import Mesh, PartitionSpec, NamedSharding
    from jax.experimental.shard_map import shard_map
    from concourse import bass2jax, mybir
    from concourse.bass2jax import _bass_exec_p, partition_id_tensor

    bass2jax.install_neuronx_cc_hook()
    n_cores = len(in_maps)
    partition_name = (nc.partition_id_tensor.name
                      if nc.partition_id_tensor else None)
    in_names, out_names, out_avals, zero_outs = [], [], [], []
    for alloc in nc.m.functions[0].allocations:
        if not isinstance(alloc, mybir.MemoryLocationSet):
            continue
        name = alloc.memorylocations[0].name
        if alloc.kind == "ExternalInput":
            if name != partition_name:
                in_names.append(name)
        elif alloc.kind == "ExternalOutput":
            shape = tuple(alloc.tensor_shape)
            dtype = mybir.dt.np(alloc.dtype)
            out_names.append(name)
            out_avals.append(jax.core.ShapedArray(shape, dtype))
            zero_outs.append(np.zeros(shape, dtype))
    n_params = len(in_names)
    n_outs = len(out_avals)
    param_names = list(in_names)
    in_names = in_names + out_names
    if partition_name is not None:
        in_names.append(partition_name)

    def _body(*args):
        operands = list(args)
        if partition_name is not None:
            operands.append(partition_id_tensor())
        outs = _bass_exec_p.bind(
            *operands, out_avals=tuple(out_avals), in_names=tuple(in_names),
            out_names=tuple(out_names), lowering_input_output_aliases=(),
            sim_require_finite=True, sim_require_nnan=True, nc=nc)
        return tuple(outs)

    devices = jax.devices()[:n_cores]
    mesh = Mesh(np.asarray(devices), ("core",))
    spec = PartitionSpec("core")
    sharded = jax.jit(
        shard_map(_body, mesh=mesh, in_specs=(spec,) * (n_params + n_outs),
                  out_specs=(spec,) * n_outs, check_rep=False),
        donate_argnums=tuple(range(n_params, n_params + n_outs)),
        keep_unused=True)
    sh = NamedSharding(mesh, spec)
    concat_in = [
        jax.device_put(np.concatenate(
            [np.asarray(mm[nm]) for mm in in_maps], axis=0), sh)
        for nm in param_names
    ]
    zglobal = [np.zeros((n_cores * z.shape[0], *z.shape[1:]), z.dtype)
               for z in zero_outs]

    def run_once():
        zs = [jax.device_put(z, sh) for z in zglobal]
        import time as _t
        jax.block_until_ready(zs)
        t0 = _t.perf_counter()
        out = sharded(*concat_in, *zs)
        jax.block_until_ready(out)
        dtv = _t.perf_counter() - t0
        return out, dtv

    def unpack(out):
        return [
            {nm: np.asarray(out[i]).reshape(n_cores, *out_avals[i].shape)[c]
             for i, nm in enumerate(out_names)}
            for c in range(n_cores)
        ]

    return run_once, unpack


def _floor_est(ts, band=2e-3):
    """Robust floor: median of the fast-mode cluster (within `band` of min).
    The axon tunnel overhead is strongly bimodal with a drifting floor; the
    cluster median is much stabler than a bare min."""
    s = sorted(ts)
    cl = [x for x in s if x <= s[0] + band]
    return cl[len(cl) // 2]


def bench(inputs, iters=6, rep=17):
    """HW timing via on-device amplification: the same program emitted once
    vs `rep` times back-to-back; (T_rep - T_1)/(rep-1) cancels the axon
    tunnel overhead (~30-90ms/call).  Interleaved sampling shares the noise
    environment between the two variants."""
    sched = _schedule(inputs["lengths"])
    maps = _in_maps(inputs, sched)
    nc1, _ = _build(inputs["lengths"], 1)
    run1, unpack1 = _timed_runner(nc1, maps)
    ncR, _ = _build(inputs["lengths"], rep)
    runR, _ = _timed_runner(ncR, maps)
    t1s, tRs = [], []
    out = None
    run1(); runR()  # warmup
    for _ in range(max(iters, 40)):
        out, dt1 = run1()
        _, dtR = runR()
        t1s.append(dt1)
        tRs.append(dtR)
    raw = [r["preds"] for r in unpack1(out)]
    preds = _postprocess(raw, inputs, sched)
    est = (_floor_est(tRs) - _floor_est(t1s)) / (rep - 1) * 1e9
    print(f"[bench] rep1 walls (ms): {[round(t*1e3,2) for t in t1s]}")
    print(f"[bench] rep{rep} walls (ms): {[round(t*1e3,2) for t in tRs]}")
    return preds, int(est)
